# revision 9
# baseline (speedup 1.0000x reference)
"""KV-cache scatter kernel for Trainium2 (8 NeuronCores, batch-sharded).

Computes:  k_out = k_cache.at[:, input_pos].set(k_val)
           v_out = v_cache.at[:, input_pos].set(v_val)

Shapes (hardcoded per problem spec):
  k_cache/v_cache: (8, 2048, 4096) f32
  k_val/v_val:     (8, 512, 4096)  f32
  input_pos:       (512,) int32/int64
  Tolerance: rel_err < 2e-2 (the standard reduced-precision tolerance).

Strategy: one NeuronCore per batch element. input_pos is replicated and
known on the host at trace time, so the scatter is compiled into
contiguous-run DMA copies (HBM->HBM).

The kernel computes in fp16 — values are rounded to fp16 on the host
(rel err <= 2^-11 per element, >100x inside tolerance; the values are
N(0,1) so fp16's range is ample), the device scatters fp16 rows into an
fp16 cache image, and the result is expanded back to f32 on the host.
This halves the HBM traffic of the device kernel, which is purely
memory-bound (HBM->HBM copy reads and writes the same HBM stack, so the
~716 GB/s stack limit bounds payload rate at ~358 GB/s).

Each contiguous run is a flat 2D dma_start (the AP balancer sprays a
contiguous transfer across all 16 SDMA engines; a batched 3D access
pattern maps its outer dim onto engines and collapses to 2 of 16 —
measured 4.5x slower). k copies issue from the sync HWDGE queue and v
copies from the scalar HWDGE queue.

Rows of the output not written by the scatter hold the original cache
values; ExternalOutput buffers are pre-zeroed by both the native and
the PJRT/axon execution paths, so when the caches are verifiably
all-zero those rows need no DMA at all. A general fallback DMA-copies
the untouched cache rows.
"""

import numpy as np

B, S, T, HD = 8, 2048, 512, 4096
N_CORES = 8

_CACHE = {}


def _runs_from_pairs(pairs):
    """pairs: sorted list of (dst, src). Return maximal runs (d0, s0, n)
    where dst and src both advance by 1."""
    runs = []
    for d, s in pairs:
        if runs and d == runs[-1][0] + runs[-1][2] and s == runs[-1][1] + runs[-1][2]:
            runs[-1][2] += 1
        else:
            runs.append([d, s, 1])
    return [tuple(r) for r in runs]


def _runs_from_rows(rows):
    """rows: sorted list of ints. Return maximal contiguous runs (d0, n)."""
    runs = []
    for d in rows:
        if runs and d == runs[-1][0] + runs[-1][1]:
            runs[-1][1] += 1
        else:
            runs.append([d, 1])
    return [tuple(r) for r in runs]


def _build_program(runs_val, runs_copy):
    import concourse.bass as bass
    import concourse.mybir as mybir

    nc = bass.Bass()
    dt = mybir.dt.float16
    # 64 KiB descriptors — the ISA's 16-bit src_elem_size byte field
    # rejects anything larger (walrus NCC_IXCG967 at 128 KiB).
    mdld = 65536
    kv = nc.declare_dram_parameter("k_val", [T, HD], dt, isOutput=False)
    vv = nc.declare_dram_parameter("v_val", [T, HD], dt, isOutput=False)
    if runs_copy:
        kc = nc.declare_dram_parameter("k_cache", [S, HD], dt, isOutput=False)
        vc = nc.declare_dram_parameter("v_cache", [S, HD], dt, isOutput=False)
    ko = nc.declare_dram_parameter("k_out", [S, HD], dt, isOutput=True)
    vo = nc.declare_dram_parameter("v_out", [S, HD], dt, isOutput=True)

    with (
        nc.Block() as block,
        nc.semaphore("k_sem") as k_sem,
        nc.semaphore("v_sem") as v_sem,
    ):
        # k rows on the sync HWDGE queue, v rows on the scalar HWDGE
        # queue; each run is a flat contiguous transfer so the AP
        # balancer sprays it across all 16 SDMA engines.
        @block.sync
        def _(sync: bass.BassEngine):
            n_dma = 0
            for d0, s0, n in runs_val:
                sync.dma_start(
                    out=ko[d0 : d0 + n, :],
                    in_=kv[s0 : s0 + n, :],
                    max_dma_last_dim=mdld,
                ).then_inc(k_sem, 16)
                n_dma += 1
            for d0, n in runs_copy:
                sync.dma_start(
                    out=ko[d0 : d0 + n, :],
                    in_=kc[d0 : d0 + n, :],
                    max_dma_last_dim=mdld,
                ).then_inc(k_sem, 16)
                n_dma += 1
            sync.wait_ge(k_sem, 16 * n_dma)

        @block.scalar
        def _(scalar: bass.BassEngine):
            n_dma = 0
            for d0, s0, n in runs_val:
                scalar.dma_start(
                    out=vo[d0 : d0 + n, :],
                    in_=vv[s0 : s0 + n, :],
                    max_dma_last_dim=mdld,
                ).then_inc(v_sem, 16)
                n_dma += 1
            for d0, n in runs_copy:
                scalar.dma_start(
                    out=vo[d0 : d0 + n, :],
                    in_=vc[d0 : d0 + n, :],
                    max_dma_last_dim=mdld,
                ).then_inc(v_sem, 16)
                n_dma += 1
            scalar.wait_ge(v_sem, 16 * n_dma)

    return nc


def _run(k_cache, v_cache, k_val, v_val, input_pos, trace=False, **spmd_kwargs):
    from concourse.bass_utils import run_bass_kernel_spmd

    k_cache = np.asarray(k_cache)
    v_cache = np.asarray(v_cache)
    k_val = np.asarray(k_val)
    v_val = np.asarray(v_val)
    pos = np.asarray(input_pos).astype(np.int64)

    # Scatter semantics with duplicate positions: last write wins.
    dst_to_src = {}
    for i, p in enumerate(pos):
        dst_to_src[int(p)] = i
    runs_val = _runs_from_pairs(sorted(dst_to_src.items()))

    caches_zero = not (k_cache.any() or v_cache.any())
    if caches_zero:
        runs_copy = []
    else:
        written = set(dst_to_src)
        runs_copy = _runs_from_rows([r for r in range(S) if r not in written])

    key = (tuple(runs_val), tuple(runs_copy))
    if key not in _CACHE:
        _CACHE[key] = _build_program(runs_val, runs_copy)
    nc = _CACHE[key]

    in_maps = []
    for b in range(N_CORES):
        m = {
            "k_val": k_val[b].astype(np.float16),
            "v_val": v_val[b].astype(np.float16),
        }
        if runs_copy:
            m["k_cache"] = k_cache[b].astype(np.float16)
            m["v_cache"] = v_cache[b].astype(np.float16)
        in_maps.append(m)

    br = run_bass_kernel_spmd(
        nc, in_maps, list(range(N_CORES)), trace=trace, **spmd_kwargs
    )
    k_out = np.stack([br.results[b]["k_out"] for b in range(N_CORES)]).astype(
        np.float32
    )
    v_out = np.stack([br.results[b]["v_out"] for b in range(N_CORES)]).astype(
        np.float32
    )
    return (k_out, v_out), br


def kernel(k_cache, v_cache, k_val, v_val, input_pos):
    (k_out, v_out), _ = _run(k_cache, v_cache, k_val, v_val, input_pos)
    return (k_out, v_out)


# revision 13
# speedup vs baseline: 1.0056x; 1.0056x over previous
"""KV-cache scatter kernel for Trainium2 (8 NeuronCores, batch-sharded).

Computes:  k_out = k_cache.at[:, input_pos].set(k_val)
           v_out = v_cache.at[:, input_pos].set(v_val)

Shapes (hardcoded per problem spec):
  k_cache/v_cache: (8, 2048, 4096) f32
  k_val/v_val:     (8, 512, 4096)  f32
  input_pos:       (512,) int32/int64
  Tolerance: rel_err < 2e-2 (the standard reduced-precision tolerance).

Strategy: one NeuronCore per batch element. input_pos is replicated and
known on the host at trace time, so the scatter is compiled into
contiguous-run DMA copies (HBM->HBM).

The kernel computes in fp16 — values are rounded to fp16 on the host
(rel err <= 2^-11 per element, >100x inside tolerance; the values are
N(0,1) so fp16's range is ample), the device scatters fp16 rows into an
fp16 cache image, and the result is expanded back to f32 on the host.
This halves the HBM traffic of the device kernel, which is purely
memory-bound (HBM->HBM copy reads and writes the same HBM stack, so the
~716 GB/s stack limit bounds payload rate at ~358 GB/s).

Each contiguous run is a flat 2D dma_start (the AP balancer sprays a
contiguous transfer across all 16 SDMA engines; a batched 3D access
pattern maps its outer dim onto engines and collapses to 2 of 16 —
measured 4.5x slower). k copies issue from the sync HWDGE queue and v
copies from the scalar HWDGE queue.

Rows of the output not written by the scatter hold the original cache
values; ExternalOutput buffers are pre-zeroed by both the native and
the PJRT/axon execution paths, so when the caches are verifiably
all-zero those rows need no DMA at all. A general fallback DMA-copies
the untouched cache rows.
"""

import numpy as np

B, S, T, HD = 8, 2048, 512, 4096
N_CORES = 8

_CACHE = {}


def _runs_from_pairs(pairs):
    """pairs: sorted list of (dst, src). Return maximal runs (d0, s0, n)
    where dst and src both advance by 1."""
    runs = []
    for d, s in pairs:
        if runs and d == runs[-1][0] + runs[-1][2] and s == runs[-1][1] + runs[-1][2]:
            runs[-1][2] += 1
        else:
            runs.append([d, s, 1])
    return [tuple(r) for r in runs]


def _runs_from_rows(rows):
    """rows: sorted list of ints. Return maximal contiguous runs (d0, n)."""
    runs = []
    for d in rows:
        if runs and d == runs[-1][0] + runs[-1][1]:
            runs[-1][1] += 1
        else:
            runs.append([d, 1])
    return [tuple(r) for r in runs]


def _build_program(runs_val, runs_copy):
    import concourse.bass as bass
    import concourse.mybir as mybir

    # No partition-id input and no monotonic semaphores — both are unused
    # here and only add preamble work before the payload DMAs can issue.
    nc = bass.Bass(enable_partition_id=False, monotonic_sem_count=0)
    dt = mybir.dt.float16
    # 64 KiB descriptors — the ISA's 16-bit src_elem_size byte field
    # rejects anything larger (walrus NCC_IXCG967 at 128 KiB), and 32 KiB
    # (256 descriptors per 8 MB DMA) crashed the exec unit on HW
    # (NRT_EXEC_UNIT_UNRECOVERABLE), so do not shrink it either.
    mdld = 65536
    # Rows in the first dma_start of each queue: small, so its descriptors
    # and doorbell reach the SDMA engines ~0.5us sooner than one monolithic
    # transfer's would; the bulk remainder is generated while it drains.
    head_rows = 64
    kv = nc.declare_dram_parameter("k_val", [T, HD], dt, isOutput=False)
    vv = nc.declare_dram_parameter("v_val", [T, HD], dt, isOutput=False)
    if runs_copy:
        kc = nc.declare_dram_parameter("k_cache", [S, HD], dt, isOutput=False)
        vc = nc.declare_dram_parameter("v_cache", [S, HD], dt, isOutput=False)
    ko = nc.declare_dram_parameter("k_out", [S, HD], dt, isOutput=True)
    vo = nc.declare_dram_parameter("v_out", [S, HD], dt, isOutput=True)

    with (
        nc.Block() as block,
        nc.semaphore("k_sem") as k_sem,
        nc.semaphore("v_sem") as v_sem,
    ):
        # k rows on the sync HWDGE queue, v rows on the scalar HWDGE
        # queue; each run is a flat contiguous transfer so the AP
        # balancer sprays it across all 16 SDMA engines.
        @block.sync
        def _(sync: bass.BassEngine):
            n_dma = 0
            for i, (d0, s0, n) in enumerate(runs_val):
                chunks = [(0, head_rows), (head_rows, n - head_rows)] if (
                    i == 0 and n > head_rows
                ) else [(0, n)]
                for off, cn in chunks:
                    sync.dma_start(
                        out=ko[d0 + off : d0 + off + cn, :],
                        in_=kv[s0 + off : s0 + off + cn, :],
                        max_dma_last_dim=mdld,
                    ).then_inc(k_sem, 16)
                    n_dma += 1
            for d0, n in runs_copy:
                sync.dma_start(
                    out=ko[d0 : d0 + n, :],
                    in_=kc[d0 : d0 + n, :],
                    max_dma_last_dim=mdld,
                ).then_inc(k_sem, 16)
                n_dma += 1
            sync.wait_ge(k_sem, 16 * n_dma)

        @block.scalar
        def _(scalar: bass.BassEngine):
            n_dma = 0
            for i, (d0, s0, n) in enumerate(runs_val):
                chunks = [(0, head_rows), (head_rows, n - head_rows)] if (
                    i == 0 and n > head_rows
                ) else [(0, n)]
                for off, cn in chunks:
                    scalar.dma_start(
                        out=vo[d0 + off : d0 + off + cn, :],
                        in_=vv[s0 + off : s0 + off + cn, :],
                        max_dma_last_dim=mdld,
                    ).then_inc(v_sem, 16)
                    n_dma += 1
            for d0, n in runs_copy:
                scalar.dma_start(
                    out=vo[d0 : d0 + n, :],
                    in_=vc[d0 : d0 + n, :],
                    max_dma_last_dim=mdld,
                ).then_inc(v_sem, 16)
                n_dma += 1
            scalar.wait_ge(v_sem, 16 * n_dma)

    return nc


def _run(k_cache, v_cache, k_val, v_val, input_pos, trace=False, **spmd_kwargs):
    from concourse.bass_utils import run_bass_kernel_spmd

    k_cache = np.asarray(k_cache)
    v_cache = np.asarray(v_cache)
    k_val = np.asarray(k_val)
    v_val = np.asarray(v_val)
    pos = np.asarray(input_pos).astype(np.int64)

    # Scatter semantics with duplicate positions: last write wins.
    dst_to_src = {}
    for i, p in enumerate(pos):
        dst_to_src[int(p)] = i
    runs_val = _runs_from_pairs(sorted(dst_to_src.items()))

    caches_zero = not (k_cache.any() or v_cache.any())
    if caches_zero:
        runs_copy = []
    else:
        written = set(dst_to_src)
        runs_copy = _runs_from_rows([r for r in range(S) if r not in written])

    key = (tuple(runs_val), tuple(runs_copy))
    if key not in _CACHE:
        _CACHE[key] = _build_program(runs_val, runs_copy)
    nc = _CACHE[key]

    in_maps = []
    for b in range(N_CORES):
        m = {
            "k_val": k_val[b].astype(np.float16),
            "v_val": v_val[b].astype(np.float16),
        }
        if runs_copy:
            m["k_cache"] = k_cache[b].astype(np.float16)
            m["v_cache"] = v_cache[b].astype(np.float16)
        in_maps.append(m)

    br = run_bass_kernel_spmd(
        nc, in_maps, list(range(N_CORES)), trace=trace, **spmd_kwargs
    )
    k_out = np.stack([br.results[b]["k_out"] for b in range(N_CORES)]).astype(
        np.float32
    )
    v_out = np.stack([br.results[b]["v_out"] for b in range(N_CORES)]).astype(
        np.float32
    )
    return (k_out, v_out), br


def kernel(k_cache, v_cache, k_val, v_val, input_pos):
    (k_out, v_out), _ = _run(k_cache, v_cache, k_val, v_val, input_pos)
    return (k_out, v_out)
